# revision 10
# baseline (speedup 1.0000x reference)
"""AttentionSequencePoolingLayer on 8 TRN2 NeuronCores (Bass/Tile).

Math (per batch b):
  att_in = [q, k, q-k, q*k] @ W1 + b1  ->  sigmoid -> @W2+b2 -> sigmoid -> @W3+b3
  scores masked -> softmax over T -> attn @ keys

Folding: att_in @ W1 = k @ Weff_b + c_b, with
  Weff_b = (W1k - W1m) + q_b * W1p   (per-batch effective weight, [64,8])
  c_b    = q_b @ (W1q + W1m) + b1    (per-batch bias, [8])
where W1 = [W1q; W1k; W1m; W1p] row-blocks. b3 drops out (softmax shift-invariant).

Device layout (per core, 512 batches, data-parallel over batch):
  - keys loaded via SWDGE cast-DMA (f32->bf16), 8 batches per DMA, 512B/partition
    lines: super-tile [100, 1024], partition p holds rows {2p, 2p+1} of each batch.
  - PE transpose per batch [100,128] -> psum [128,100] = kT (even-t rows on
    partitions 0:64, odd-t on 64:128).  t-permutation: col j<100 <-> t=2j,
    col 100+j <-> t=2j+1 (masks pre-permuted on host to match).
  - quad copies assemble kTq [128, 2x200]: pair layout (even batch of pair on
    partitions 0:64, odd batch on 64:128).
  - mm1: block-diag Weff pair [128,16] x kTq -> ps1 [16,200] per pair, 4 pairs
    stacked at 32-aligned strips -> sigmoid(+c) -> h1T [128,200] bf16 (8 batches).
  - mm2: block-diag W2 [128,32] -> ps2 strips (32 batches/tile) -> sigmoid -> h2T.
  - mm3: block-diag W3 [128,32] -> ps3 [128,200] = scoresT for 128 batches, dense.
  - softmax along free dim with mask-select, exp row-sums via ACT accum_out.
  - attn transposed back (PE) -> attn columns bf16; pooling = per batch two
    N=64 matmuls (attn column stationary) accumulating [1,64] psum slots.
"""
import os
import sys
import numpy as np

for _p in ("/opt/trn_rl_repo",):
    if os.path.isdir(_p) and _p not in sys.path:
        sys.path.insert(0, _p)

import ml_dtypes  # noqa: E402
from contextlib import ExitStack  # noqa: E402
import concourse.bass as bass  # noqa: E402
import concourse.tile as tile  # noqa: E402
from concourse import bacc, mybir  # noqa: E402
from concourse.bass_utils import run_bass_kernel_spmd  # noqa: E402

B, T, D = 4096, 200, 64
NCORES = 8
BC = B // NCORES            # 512 batches per core
NEG = np.float32(-2**32 + 1)
BF16 = mybir.dt.bfloat16
F32 = mybir.dt.float32
U8 = mybir.dt.uint8
TT_BATCHES = 64             # batches per "tile" (softmax super-group)
NTT = BC // TT_BATCHES      # 4


def _build_kernel(ntt=NTT):
    """Build the Bass program. ntt = number of 128-batch tiles (4 = full)."""
    nbatch = ntt * TT_BATCHES
    nc = bacc.Bacc("TRN2", target_bir_lowering=False, debug=False,
                   num_devices=NCORES)
    keys_d = nc.dram_tensor("keys", [nbatch, T, D], BF16, kind="ExternalInput").ap()
    weff_d = nc.dram_tensor("weff", [128, nbatch * 32], BF16, kind="ExternalInput").ap()
    cbias_d = nc.dram_tensor("cbias", [128, nbatch // 4], F32, kind="ExternalInput").ap()
    bd2_d = nc.dram_tensor("bd2", [128, 32], BF16, kind="ExternalInput").ap()
    bd3e_d = nc.dram_tensor("bd3e", [128, 32], BF16, kind="ExternalInput").ap()
    bd3o_d = nc.dram_tensor("bd3o", [128, 32], BF16, kind="ExternalInput").ap()
    b2rep_d = nc.dram_tensor("b2rep", [128, 1], F32, kind="ExternalInput").ap()
    masks_d = nc.dram_tensor("masks", [nbatch * 2, T], U8, kind="ExternalInput").ap()
    neg_d = nc.dram_tensor("negt", [128, T], F32, kind="ExternalInput").ap()
    id16_d = nc.dram_tensor("id16", [128, 128], BF16, kind="ExternalInput").ap()
    idf_d = nc.dram_tensor("idf", [128, 128], F32, kind="ExternalInput").ap()
    out_d = nc.dram_tensor("out", [nbatch, D], F32, kind="ExternalOutput").ap()

    with tile.TileContext(nc) as tc:
        with ExitStack() as ctx:
            _body(ctx, tc, ntt, keys_d, weff_d, cbias_d, bd2_d, bd3e_d, bd3o_d,
                  b2rep_d, masks_d, neg_d, id16_d, idf_d, out_d)
    nc.compile()
    return nc


def _body(ctx, tc, ntt, keys_d, weff_d, cbias_d, bd2_d, bd3e_d, bd3o_d,
          b2rep_d, masks_d, neg_d, id16_d, idf_d, out_d):
    nc = tc.nc
    Sig = mybir.ActivationFunctionType.Sigmoid
    Exp = mybir.ActivationFunctionType.Exp
    X = mybir.AxisListType.X
    MAX = mybir.AluOpType.max

    const = ctx.enter_context(tc.tile_pool(name="const", bufs=1))
    sup_pool = ctx.enter_context(tc.tile_pool(name="sup", bufs=20))
    ktq_pool = ctx.enter_context(tc.tile_pool(name="ktq", bufs=4))
    h1_pool = ctx.enter_context(tc.tile_pool(name="h1", bufs=3))
    h2_pool = ctx.enter_context(tc.tile_pool(name="h2", bufs=2))
    sm_pool = ctx.enter_context(tc.tile_pool(name="sm", bufs=2))
    at_pool = ctx.enter_context(tc.tile_pool(name="at", bufs=2))
    msk_pool = ctx.enter_context(tc.tile_pool(name="msk", bufs=2))
    st_pool = ctx.enter_context(tc.tile_pool(name="st", bufs=2))
    ps_t = ctx.enter_context(tc.tile_pool(name="pst", bufs=2, space="PSUM"))
    ps_1 = ctx.enter_context(tc.tile_pool(name="ps1", bufs=1, space="PSUM"))
    ps_2 = ctx.enter_context(tc.tile_pool(name="ps2", bufs=1, space="PSUM"))
    ps_3 = ctx.enter_context(tc.tile_pool(name="ps3", bufs=1, space="PSUM"))
    ps_a = ctx.enter_context(tc.tile_pool(name="psa", bufs=1, space="PSUM"))
    ps_p = ctx.enter_context(tc.tile_pool(name="psp", bufs=2, space="PSUM"))

    # constants
    weff = const.tile([128, weff_d.shape[1]], BF16)
    nc.sync.dma_start(weff[:], weff_d[:])
    cbias = const.tile([128, cbias_d.shape[1]], F32)
    nc.sync.dma_start(cbias[:], cbias_d[:])
    bd2 = const.tile([128, 32], BF16)
    nc.sync.dma_start(bd2[:], bd2_d[:])
    bd3e = const.tile([128, 32], BF16)
    nc.sync.dma_start(bd3e[:], bd3e_d[:])
    bd3o = const.tile([128, 32], BF16)
    nc.sync.dma_start(bd3o[:], bd3o_d[:])
    b2rep = const.tile([128, 1], F32)
    nc.sync.dma_start(b2rep[:], b2rep_d[:])
    negt = const.tile([128, T], F32)
    nc.sync.dma_start(negt[:], neg_d[:])
    id16 = const.tile([128, 128], BF16)
    nc.sync.dma_start(id16[:], id16_d[:])
    idf = const.tile([128, 128], F32)
    nc.sync.dma_start(idf[:], idf_d[:])

    # keys dram view: [nbatch, T, D] -> per 8-batch super-tile [100, 8*128]
    # dst col = 128*bb + 64*r + d ; src row = 2p + r
    keys_r = keys_d.rearrange("b (p r) d -> b p r d", r=2)  # [nb, 100, 2, 64]

    for tt in range(ntt):
        b0 = tt * TT_BATCHES
        supers = []
        for s8 in range(TT_BATCHES // 8):
            sup = sup_pool.tile([128, 1024], BF16, tag="sup")
            src = keys_r[b0 + s8 * 8: b0 + s8 * 8 + 8]  # [8,100,2,64]
            # dst [100, (bb r d)] <- src[bb, p, r, d]
            dst = sup[0:100, :].rearrange("p (bb r d) -> p bb r d", bb=8, r=2)
            eng = (nc.sync, nc.gpsimd, nc.scalar, nc.gpsimd)[s8 % 4]
            eng.dma_start(dst, src.rearrange("bb p r d -> p bb r d"))
            supers.append(sup)

        mask_t = msk_pool.tile([128, T], U8, tag="msk")
        nc.gpsimd.dma_start(mask_t[:], masks_d[2 * b0:2 * b0 + 128, :])

        ps3 = ps_3.tile([128, T], F32, tag="ps3")
        for w16 in range(4):          # 16-batch block -> two mm3 (even/odd)
            ps2 = ps_2.tile([128, 100], F32, tag="ps2")
            for g4 in range(4):       # 4-batch group (one quad) -> sig1 + mm2
                qb = b0 + w16 * 16 + g4 * 4          # first batch of quad
                ps1 = ps_1.tile([128, 100], F32, tag="ps1")
                pst = ps_t.tile([128, 400], BF16, tag="pst")
                for bq in range(4):
                    bl = qb - b0 + bq      # batch offset within tile
                    sup = supers[bl // 8]
                    cb = 128 * (bl % 8)
                    nc.tensor.transpose(
                        pst[:, 100 * bq:100 * (bq + 1)],
                        sup[0:100, cb:cb + 128], id16[0:100, 0:100])
                ktq = ktq_pool.tile([128, 400], BF16, tag="ktq")
                if (qb // 4) % 2 == 0:
                    nc.scalar.copy(ktq[:], pst[:])
                else:
                    nc.vector.tensor_copy(ktq[:], pst[:])
                for bq in range(4):
                    P = qb + bq          # global batch index
                    kst = 32 * (P % 4)
                    nc.tensor.matmul(
                        ps1[kst:kst + 32, :],
                        lhsT=weff[:, 32 * P:32 * (P + 1)],
                        rhs=ktq[:, 100 * bq:100 * (bq + 1)],
                        start=True, stop=True, tile_position=(0, kst))
                G = (b0 + w16 * 16) // 4 + g4        # global 4-batch group
                h1 = h1_pool.tile([128, 100], BF16, tag="h1")
                nc.scalar.activation(h1[:], ps1[:], Sig,
                                     bias=cbias[:, G:G + 1])
                nc.tensor.matmul(ps2[32 * g4:32 * g4 + 32, :], lhsT=bd2[:],
                                 rhs=h1[:], start=True, stop=True,
                                 tile_position=(0, 32 * g4))
            h2 = h2_pool.tile([128, 100], BF16, tag="h2")
            nc.scalar.activation(h2[:], ps2[:], Sig, bias=b2rep[:])
            nc.tensor.matmul(ps3[32 * w16:32 * w16 + 32, 0:100], lhsT=bd3e[:],
                             rhs=h2[:], start=True, stop=True,
                             tile_position=(0, 32 * w16))
            nc.tensor.matmul(ps3[32 * w16:32 * w16 + 32, 100:200], lhsT=bd3o[:],
                             rhs=h2[:], start=True, stop=True,
                             tile_position=(0, 32 * w16))

        # ---- softmax over free dim (cols are t-permuted; masks match) ----
        sc = sm_pool.tile([128, T], F32, tag="sc")
        nc.vector.tensor_copy(sc[:], negt[:])
        nc.vector.copy_predicated(sc[:], mask_t[:], ps3[:])
        nmx = sm_pool.tile([128, 1], F32, tag="nmx")
        nc.vector.tensor_reduce(nmx[:], sc[:], axis=X, op=MAX, negate=True)
        e = sm_pool.tile([128, T], F32, tag="e")
        ssum = sm_pool.tile([128, 1], F32, tag="ssum")
        nc.scalar.activation(e[:], sc[:], Exp, bias=nmx[:], accum_out=ssum[:])
        rs = sm_pool.tile([128, 1], F32, tag="rs")
        nc.vector.reciprocal(rs[:], ssum[:])
        attn = sm_pool.tile([128, T], F32, tag="attn")
        nc.vector.tensor_scalar_mul(attn[:], e[:], rs[:])

        # ---- transpose attn -> columns (bf16) ----
        psa = ps_a.tile([100, 256], F32, tag="psa")
        nc.tensor.transpose(psa[0:100, 0:128], attn[:, 0:100], idf[:])
        nc.tensor.transpose(psa[0:100, 128:256], attn[:, 100:200], idf[:])
        at = at_pool.tile([100, 256], BF16, tag="at")
        nc.scalar.copy(at[:], psa[:])

        # ---- pooling: per batch two N=64 matmuls into [1,64] psum slots ----
        for bank in range(2):
            psp = ps_p.tile([128, 512], F32, tag="psp")
            nc.vector.memset(psp[:], 0.0)
            for w in range(32):
                u = bank * 32 + w
                sup = supers[u // 8]
                cb = 128 * (u % 8)
                cu = 32 * (u // 16) + (u % 16)   # sparse attn row of batch u
                part = 32 * (w % 4)
                col = 64 * (w // 4)
                nc.tensor.matmul(
                    psp[part:part + 1, col:col + 64],
                    lhsT=at[0:100, cu:cu + 1],
                    rhs=sup[0:100, cb:cb + 64],
                    start=True, stop=False, tile_position=(0, part),
                    skip_group_check=True)
                nc.tensor.matmul(
                    psp[part:part + 1, col:col + 64],
                    lhsT=at[0:100, 128 + cu:128 + cu + 1],
                    rhs=sup[0:100, cb + 64:cb + 128],
                    start=False, stop=True, tile_position=(0, part),
                    skip_group_check=True)
            stg = st_pool.tile([128, 512], F32, tag="stg")
            nc.scalar.copy(stg[:], psp[:])
            # out rows: batch b0+bank*32 + 4*s + k at stg[32k, 64s:64s+64]
            ob = b0 + bank * 32
            dst = out_d[ob:ob + 32].rearrange("(s k) d -> k s d", k=4)
            for k in range(4):
                nc.gpsimd.dma_start(dst[k:k + 1, :], stg[32 * k:32 * k + 1, :])


_NC_CACHE = {}


def _get_nc(ntt=NTT):
    if ntt not in _NC_CACHE:
        _NC_CACHE[ntt] = _build_kernel(ntt)
    return _NC_CACHE[ntt]


def make_core_inputs(queries, keys, key_masks, W1, b1, W2, b2, W3, b3,
                     core, ntt=NTT):
    """Host-side prep of one core's input map (all numpy)."""
    nb = ntt * TT_BATCHES
    cs = core * BC
    q = np.asarray(queries[cs:cs + nb, 0, :], dtype=np.float32)      # [nb,64]
    k = np.ascontiguousarray(
        np.asarray(keys[cs:cs + nb], dtype=np.float32).astype(ml_dtypes.bfloat16))
    m = np.asarray(key_masks[cs:cs + nb, 0, :])                      # [nb,200] bool
    W1 = np.asarray(W1, np.float32); W2 = np.asarray(W2, np.float32)
    W3 = np.asarray(W3, np.float32)
    b1 = np.asarray(b1, np.float32); b2 = np.asarray(b2, np.float32)
    W1q, W1k, W1m, W1p = W1[0:64], W1[64:128], W1[128:192], W1[192:256]
    Weff = (W1k - W1m)[None] + q[:, :, None] * W1p[None]             # [nb,64,8]
    c = q @ (W1q + W1m) + b1                                         # [nb,8]

    # weff per-batch parity blocks: [128, nb*32]; batch b cols 32b:32b+32
    weff = np.zeros((128, nb * 32), np.float32)
    wr = weff.reshape(128, nb, 32)
    wr[0:64, :, 0:8] = Weff.transpose(1, 0, 2)
    wr[64:128, :, 8:16] = Weff.transpose(1, 0, 2)

    # cbias [128, nb//4]: [32k+8par+j, G4] = c[4*G4+k][j], par in {0,1}
    nG = nb // 4
    cb = np.zeros((4, 4, 8, nG), np.float32)      # [k, sub(4=2par+pad?), j, G]
    ci = c.reshape(nG, 4, 8).transpose(1, 2, 0)   # [k, j, G]
    cb[:, 0, :, :] = ci
    cb[:, 1, :, :] = ci
    cb = np.ascontiguousarray(cb.reshape(128, nG))

    # BD2P [128,32]: [32k+8par+j, 4(2k+par)+cc] = W2[j,cc]
    bd2 = np.zeros((128, 32), np.float32)
    for kk in range(4):
        for par in range(2):
            mloc = 2 * kk + par
            bd2[32 * kk + 8 * par:32 * kk + 8 * par + 8,
                4 * mloc:4 * mloc + 4] = W2
    # BD3e/o [128,32]: [32j2+8k+4par+cc, 4j2+k] = W3[cc,0]
    bd3e = np.zeros((128, 32), np.float32)
    bd3o = np.zeros((128, 32), np.float32)
    for j2 in range(4):
        for kk in range(4):
            r = 32 * j2 + 8 * kk
            bd3e[r:r + 4, 4 * j2 + kk] = W3[:, 0]
            bd3o[r + 4:r + 8, 4 * j2 + kk] = W3[:, 0]
    b2r = np.zeros((128, 1), np.float32)
    for j2 in range(4):
        for mm in range(8):
            b2r[32 * j2 + 4 * mm:32 * j2 + 4 * mm + 4, 0] = b2

    # masks: t-permutation (even t then odd t), uint8
    perm = np.concatenate([np.arange(0, T, 2), np.arange(1, T, 2)])
    mperm_dense = m[:, perm].astype(np.uint8)       # [nb, 200]
    # sparse rows: row 128*tt + 32*k3 + w (w<16) = batch 64*tt + 16*k3 + w
    mperm = np.zeros((nb * 2, T), np.uint8)
    md = mperm_dense.reshape(nb // 64, 4, 16, T)    # [tt, k3, w, T]
    ms = mperm.reshape(nb // 64, 4, 32, T)
    ms[:, :, 0:16, :] = md

    negt = np.full((128, T), NEG, np.float32)
    id16 = np.eye(128, dtype=np.float32)
    return {
        "keys": k,
        "weff": weff.astype(ml_dtypes.bfloat16),
        "cbias": cb,
        "bd2": bd2.astype(ml_dtypes.bfloat16),
        "bd3e": bd3e.astype(ml_dtypes.bfloat16),
        "bd3o": bd3o.astype(ml_dtypes.bfloat16),
        "b2rep": b2r,
        "masks": mperm,
        "negt": negt,
        "id16": id16.astype(ml_dtypes.bfloat16),
        "idf": id16,
    }


def kernel(queries, keys, key_masks, W1, b1, W2, b2, W3, b3):
    nc = _get_nc(NTT)
    in_maps = [make_core_inputs(queries, keys, key_masks, W1, b1, W2, b2,
                                W3, b3, core) for core in range(NCORES)]
    res = run_bass_kernel_spmd(nc, in_maps, list(range(NCORES)))
    outs = [res.results[c]["out"] for c in range(NCORES)]
    return np.concatenate(outs, axis=0).reshape(B, 1, D).astype(np.float32)


# revision 30
# speedup vs baseline: 1.3592x; 1.3592x over previous
"""AttentionSequencePoolingLayer on 8 TRN2 NeuronCores (Bass/Tile).

Math (per batch b):
  att_in = [q, k, q-k, q*k] @ W1 + b1  ->  sigmoid -> @W2+b2 -> sigmoid -> @W3+b3
  scores masked -> softmax over T -> attn @ keys

Folding: att_in @ W1 = k @ Weff_b + c_b, with
  Weff_b = (W1k - W1m) + q_b * W1p   (per-batch effective weight, [64,8])
  c_b    = q_b @ (W1q + W1m) + b1    (per-batch bias, [8])
where W1 = [W1q; W1k; W1m; W1p] row-blocks. b3 drops out (softmax shift-invariant).

Device layout (per core, 512 batches, data-parallel over batch):
  - keys are cast to bf16 on the host (identical numerics to an on-device
    cast) and DMA'd as [100, 1024] super-tiles, 8 batches each, 512B/partition
    lines (partition p holds t-rows {2p, 2p+1}); issue spread over HWDGE+SWDGE.
  - PE transpose per batch [100,128] -> psum [128,100] = kT with even-t
    features on partitions 0:64 and odd-t on 64:128.  t-permutation:
    scores col j<100 <-> t=2j, col 100+j <-> t=2j+1 (masks host-permuted).
  - psum->sbuf is one verbatim [128,400] DVE copy per 4-batch quad (full
    128 lanes, bf16 2x mode).
  - mm1 per batch: block-diag-by-parity Weff [128,32] (M padded to 32) into
    ps1 [128,400] (16 batches), then per-group biased sigmoids -> h1 bf16.
  - mm2/mm3: block-diag W2/W3 -> ps3 [128,200] = scoresT for 64 batches
    (16 valid rows per 32-strip).
  - softmax along free dim with mask-select; exp row-sums via ACT accum_out.
  - attn transposed back (PE) -> bf16 columns; pooling = per batch two N=64
    matmuls (attn column stationary, keys moving) accumulating [1,64] psum
    slots; slots staged [128,512] -> strided out-DMAs.
"""
import os
import sys
import numpy as np

for _p in ("/opt/trn_rl_repo",):
    if os.path.isdir(_p) and _p not in sys.path:
        sys.path.insert(0, _p)

import ml_dtypes  # noqa: E402
from contextlib import ExitStack  # noqa: E402
import concourse.bass as bass  # noqa: E402
import concourse.tile as tile  # noqa: E402
from concourse import bacc, mybir  # noqa: E402
from concourse.bass_utils import run_bass_kernel_spmd  # noqa: E402

B, T, D = 4096, 200, 64
NCORES = 8
BC = B // NCORES            # 512 batches per core
NEG = np.float32(-2**32 + 1)
BF16 = mybir.dt.bfloat16
F32 = mybir.dt.float32
U8 = mybir.dt.uint8
TT_BATCHES = 64             # batches per "tile" (softmax super-group)
NTT = BC // TT_BATCHES      # 4


def _build_kernel(ntt=NTT):
    """Build the Bass program. ntt = number of 128-batch tiles (4 = full)."""
    nbatch = ntt * TT_BATCHES
    nc = bacc.Bacc("TRN2", target_bir_lowering=False, debug=False,
                   num_devices=NCORES)
    keys_d = nc.dram_tensor("keys", [nbatch, T, D], BF16, kind="ExternalInput").ap()
    weff_d = nc.dram_tensor("weff", [128, nbatch * 32], BF16, kind="ExternalInput").ap()
    cbias_d = nc.dram_tensor("cbias", [128, nbatch // 4], F32, kind="ExternalInput").ap()
    bd2_d = nc.dram_tensor("bd2", [128, 32], BF16, kind="ExternalInput").ap()
    bd3e_d = nc.dram_tensor("bd3e", [128, 32], BF16, kind="ExternalInput").ap()
    bd3o_d = nc.dram_tensor("bd3o", [128, 32], BF16, kind="ExternalInput").ap()
    b2rep_d = nc.dram_tensor("b2rep", [128, 1], F32, kind="ExternalInput").ap()
    masks_d = nc.dram_tensor("masks", [nbatch * 2, T], U8, kind="ExternalInput").ap()
    neg_d = nc.dram_tensor("negt", [128, T], F32, kind="ExternalInput").ap()
    id16_d = nc.dram_tensor("id16", [128, 128], BF16, kind="ExternalInput").ap()
    idf_d = nc.dram_tensor("idf", [128, 128], F32, kind="ExternalInput").ap()
    out_d = nc.dram_tensor("out", [nbatch, D], F32, kind="ExternalOutput").ap()

    with tile.TileContext(nc) as tc:
        with ExitStack() as ctx:
            _body(ctx, tc, ntt, keys_d, weff_d, cbias_d, bd2_d, bd3e_d, bd3o_d,
                  b2rep_d, masks_d, neg_d, id16_d, idf_d, out_d)
    nc.compile()
    return nc


def _body(ctx, tc, ntt, keys_d, weff_d, cbias_d, bd2_d, bd3e_d, bd3o_d,
          b2rep_d, masks_d, neg_d, id16_d, idf_d, out_d):
    nc = tc.nc
    Sig = mybir.ActivationFunctionType.Sigmoid
    Exp = mybir.ActivationFunctionType.Exp
    X = mybir.AxisListType.X
    MAX = mybir.AluOpType.max

    const = ctx.enter_context(tc.tile_pool(name="const", bufs=1))
    sup_pool = ctx.enter_context(tc.tile_pool(name="sup", bufs=20))
    ktq_pool = ctx.enter_context(tc.tile_pool(name="ktq", bufs=4))
    h1_pool = ctx.enter_context(tc.tile_pool(name="h1", bufs=3))
    h2_pool = ctx.enter_context(tc.tile_pool(name="h2", bufs=2))
    sm_pool = ctx.enter_context(tc.tile_pool(name="sm", bufs=2))
    at_pool = ctx.enter_context(tc.tile_pool(name="at", bufs=2))
    msk_pool = ctx.enter_context(tc.tile_pool(name="msk", bufs=2))
    st_pool = ctx.enter_context(tc.tile_pool(name="st", bufs=2))
    ps_t = ctx.enter_context(tc.tile_pool(name="pst", bufs=2, space="PSUM"))
    ps_1 = ctx.enter_context(tc.tile_pool(name="ps1", bufs=2, space="PSUM"))
    ps_2 = ctx.enter_context(tc.tile_pool(name="ps2", bufs=1, space="PSUM"))
    ps_3 = ctx.enter_context(tc.tile_pool(name="ps3", bufs=1, space="PSUM"))
    ps_a = ctx.enter_context(tc.tile_pool(name="psa", bufs=1, space="PSUM"))
    ps_p = ctx.enter_context(tc.tile_pool(name="psp", bufs=1, space="PSUM"))

    # constants
    weff = const.tile([128, weff_d.shape[1]], BF16)
    nc.sync.dma_start(weff[:], weff_d[:])
    cbias = const.tile([128, cbias_d.shape[1]], F32)
    nc.sync.dma_start(cbias[:], cbias_d[:])
    bd2 = const.tile([128, 32], BF16)
    nc.sync.dma_start(bd2[:], bd2_d[:])
    bd3e = const.tile([128, 32], BF16)
    nc.sync.dma_start(bd3e[:], bd3e_d[:])
    bd3o = const.tile([128, 32], BF16)
    nc.sync.dma_start(bd3o[:], bd3o_d[:])
    b2rep = const.tile([128, 1], F32)
    nc.sync.dma_start(b2rep[:], b2rep_d[:])
    negt = const.tile([128, T], F32)
    nc.sync.dma_start(negt[:], neg_d[:])
    id16 = const.tile([128, 128], BF16)
    nc.sync.dma_start(id16[:], id16_d[:])
    idf = const.tile([128, 128], F32)
    nc.sync.dma_start(idf[:], idf_d[:])

    # keys dram view: [nbatch, T, D] -> per 8-batch super-tile [100, 8*128]
    # dst col = 128*bb + 64*r + d ; src row = 2p + r
    keys_r = keys_d.rearrange("b (p r) d -> b p r d", r=2)  # [nb, 100, 2, 64]

    for tt in range(ntt):
        b0 = tt * TT_BATCHES
        supers = []
        for s8 in range(TT_BATCHES // 8):
            sup = sup_pool.tile([128, 1024], BF16, tag="sup")
            src = keys_r[b0 + s8 * 8: b0 + s8 * 8 + 8]  # [8,100,2,64]
            # dst [100, (bb r d)] <- src[bb, p, r, d]
            dst = sup[0:100, :].rearrange("p (bb r d) -> p bb r d", bb=8, r=2)
            eng = (nc.sync, nc.scalar, nc.sync, nc.scalar)[s8 % 4]
            eng.dma_start(dst, src.rearrange("bb p r d -> p bb r d"))
            supers.append(sup)

        mask_t = msk_pool.tile([128, T], U8, tag="msk")
        nc.gpsimd.dma_start(mask_t[:], masks_d[2 * b0:2 * b0 + 128, :])

        ps3 = ps_3.tile([128, T], F32, tag="ps3")
        ps2 = None
        for w16 in range(4):          # 16-batch block
            if w16 % 2 == 0:
                ps2 = ps_2.tile([128, 200], F32, tag="ps2")
            ps1 = ps_1.tile([128, 400], F32, tag="ps1")
            for g4 in range(4):       # 4-batch group (one quad)
                qb = b0 + w16 * 16 + g4 * 4          # first batch of quad
                pst = ps_t.tile([128, 400], BF16, tag="pst")
                for bq in range(4):
                    bl = qb - b0 + bq      # batch offset within tile
                    sup = supers[bl // 8]
                    cb = 128 * (bl % 8)
                    nc.tensor.transpose(
                        pst[:, 100 * bq:100 * (bq + 1)],
                        sup[0:100, cb:cb + 128], id16[0:100, 0:100])
                ktq = ktq_pool.tile([128, 400], BF16, tag="ktq")
                nc.vector.tensor_copy(ktq[:], pst[:])
                for bq in range(4):
                    P = qb + bq          # global batch index; strip = bq
                    sl = ps1[32 * bq:32 * bq + 32, 100 * g4:100 * (g4 + 1)]
                    nc.tensor.matmul(
                        sl, lhsT=weff[:, 32 * P:32 * (P + 1)],
                        rhs=ktq[:, 100 * bq:100 * (bq + 1)],
                        start=True, stop=True, tile_position=(0, 32 * bq))
            h1 = h1_pool.tile([128, 400], BF16, tag="h1")
            for g4 in range(4):
                G4 = (b0 + w16 * 16) // 4 + g4
                nc.scalar.activation(h1[:, 100 * g4:100 * (g4 + 1)],
                                     ps1[:, 100 * g4:100 * (g4 + 1)], Sig,
                                     bias=cbias[:, G4:G4 + 1])
            for g4 in range(4):
                nc.tensor.matmul(
                    ps2[32 * g4:32 * g4 + 32, 100 * (w16 % 2):100 * (w16 % 2) + 100],
                    lhsT=bd2[:], rhs=h1[:, 100 * g4:100 * (g4 + 1)],
                    start=True, stop=True, tile_position=(0, 32 * g4))
            if w16 % 2 == 1:
                h2 = h2_pool.tile([128, 200], BF16, tag="h2")
                nc.scalar.activation(h2[:], ps2[:], Sig, bias=b2rep[:])
                for h in range(2):
                    w16h = w16 - 1 + h
                    nc.tensor.matmul(
                        ps3[32 * w16h:32 * w16h + 32, 0:100], lhsT=bd3e[:],
                        rhs=h2[:, 100 * h:100 * h + 100], start=True, stop=True,
                        tile_position=(0, 32 * w16h))
                    nc.tensor.matmul(
                        ps3[32 * w16h:32 * w16h + 32, 100:200], lhsT=bd3o[:],
                        rhs=h2[:, 100 * h:100 * h + 100], start=True, stop=True,
                        tile_position=(0, 32 * w16h))

        # ---- softmax over free dim (cols are t-permuted; masks match) ----
        sc = sm_pool.tile([128, T], F32, tag="sc")
        nc.vector.tensor_copy(sc[:], negt[:])
        nc.vector.copy_predicated(sc[:], mask_t[:], ps3[:])
        nmx = sm_pool.tile([128, 1], F32, tag="nmx")
        nc.vector.tensor_reduce(nmx[:], sc[:], axis=X, op=MAX, negate=True)
        e = sm_pool.tile([128, T], F32, tag="e")
        ssum = sm_pool.tile([128, 1], F32, tag="ssum")
        nc.scalar.activation(e[:], sc[:], Exp, bias=nmx[:], accum_out=ssum[:])
        rs = sm_pool.tile([128, 1], F32, tag="rs")
        nc.vector.reciprocal(rs[:], ssum[:])
        attn = sm_pool.tile([128, T], F32, tag="attn")
        nc.vector.tensor_scalar_mul(attn[:], e[:], rs[:])

        # ---- transpose attn -> columns (bf16) ----
        psa = ps_a.tile([100, 256], F32, tag="psa")
        nc.tensor.transpose(psa[0:100, 0:128], attn[:, 0:100], idf[:])
        nc.tensor.transpose(psa[0:100, 128:256], attn[:, 100:200], idf[:])
        at = at_pool.tile([100, 256], BF16, tag="at")
        nc.vector.tensor_copy(at[:], psa[:])

        # ---- pooling: per batch two N=64 matmuls into [1,64] psum slots ----
        for bank in range(2):
            psp = ps_p.tile([128, 512], F32, tag="psp")
            nc.vector.memset(psp[:], 0.0)
            for w in range(32):
                u = bank * 32 + w
                sup = supers[u // 8]
                cb = 128 * (u % 8)
                cu = 32 * (u // 16) + (u % 16)   # sparse attn row of batch u
                part = 32 * (w % 4)
                col = 64 * (w // 4)
                nc.tensor.matmul(
                    psp[part:part + 1, col:col + 64],
                    lhsT=at[0:100, cu:cu + 1],
                    rhs=sup[0:100, cb:cb + 64],
                    start=True, stop=False, tile_position=(0, part),
                    skip_group_check=True)
                nc.tensor.matmul(
                    psp[part:part + 1, col:col + 64],
                    lhsT=at[0:100, 128 + cu:128 + cu + 1],
                    rhs=sup[0:100, cb + 64:cb + 128],
                    start=False, stop=True, tile_position=(0, part),
                    skip_group_check=True)
            stg = st_pool.tile([128, 512], F32, tag="stg")
            nc.vector.tensor_copy(stg[:], psp[:])
            # out rows: batch b0+bank*32 + 4*s + k at stg[32k, 64s:64s+64]
            ob = b0 + bank * 32
            dst = out_d[ob:ob + 32].rearrange("(s k) d -> k s d", k=4)
            for k in range(4):
                nc.gpsimd.dma_start(dst[k:k + 1, :], stg[32 * k:32 * k + 1, :])


_NC_CACHE = {}


def _get_nc(ntt=NTT):
    if ntt not in _NC_CACHE:
        _NC_CACHE[ntt] = _build_kernel(ntt)
    return _NC_CACHE[ntt]


def make_core_inputs(queries, keys, key_masks, W1, b1, W2, b2, W3, b3,
                     core, ntt=NTT):
    """Host-side prep of one core's input map (all numpy)."""
    nb = ntt * TT_BATCHES
    cs = core * BC
    q = np.asarray(queries[cs:cs + nb, 0, :], dtype=np.float32)      # [nb,64]
    k = np.ascontiguousarray(
        np.asarray(keys[cs:cs + nb], dtype=np.float32).astype(ml_dtypes.bfloat16))
    m = np.asarray(key_masks[cs:cs + nb, 0, :])                      # [nb,200] bool
    W1 = np.asarray(W1, np.float32); W2 = np.asarray(W2, np.float32)
    W3 = np.asarray(W3, np.float32)
    b1 = np.asarray(b1, np.float32); b2 = np.asarray(b2, np.float32)
    W1q, W1k, W1m, W1p = W1[0:64], W1[64:128], W1[128:192], W1[192:256]
    Weff = (W1k - W1m)[None] + q[:, :, None] * W1p[None]             # [nb,64,8]
    c = q @ (W1q + W1m) + b1                                         # [nb,8]

    # weff per-batch parity blocks: [128, nb*32]; batch b cols 32b:32b+32
    weff = np.zeros((128, nb * 32), np.float32)
    wr = weff.reshape(128, nb, 32)
    wr[0:64, :, 0:8] = Weff.transpose(1, 0, 2)
    wr[64:128, :, 8:16] = Weff.transpose(1, 0, 2)

    # cbias [128, nb//4]: [32k+8par+j, G4] = c[4*G4+k][j], par in {0,1}
    nG = nb // 4
    cbv = np.zeros((4, 4, 8, nG), np.float32)
    ci = c.reshape(nG, 4, 8).transpose(1, 2, 0)
    cbv[:, 0, :, :] = ci
    cbv[:, 1, :, :] = ci
    cbv = np.ascontiguousarray(cbv.reshape(128, nG))

    # BD2P [128,32]: [32k+8par+j, 4(2k+par)+cc] = W2[j,cc]
    bd2 = np.zeros((128, 32), np.float32)
    for kk in range(4):
        for par in range(2):
            mloc = 2 * kk + par
            bd2[32 * kk + 8 * par:32 * kk + 8 * par + 8,
                4 * mloc:4 * mloc + 4] = W2
    # BD3e/o [128,32]: [32j2+8k+4par+cc, 4j2+k] = W3[cc,0]
    bd3e = np.zeros((128, 32), np.float32)
    bd3o = np.zeros((128, 32), np.float32)
    for j2 in range(4):
        for kk in range(4):
            r = 32 * j2 + 8 * kk
            bd3e[r:r + 4, 4 * j2 + kk] = W3[:, 0]
            bd3o[r + 4:r + 8, 4 * j2 + kk] = W3[:, 0]
    b2r = np.zeros((128, 1), np.float32)
    for j2 in range(4):
        for mm in range(8):
            b2r[32 * j2 + 4 * mm:32 * j2 + 4 * mm + 4, 0] = b2

    # masks: t-permutation (even t then odd t), uint8
    perm = np.concatenate([np.arange(0, T, 2), np.arange(1, T, 2)])
    mperm_dense = m[:, perm].astype(np.uint8)       # [nb, 200]
    # sparse rows: row 128*tt + 32*k3 + w (w<16) = batch 64*tt + 16*k3 + w
    mperm = np.zeros((nb * 2, T), np.uint8)
    md = mperm_dense.reshape(nb // 64, 4, 16, T)    # [tt, k3, w, T]
    ms = mperm.reshape(nb // 64, 4, 32, T)
    ms[:, :, 0:16, :] = md

    negt = np.full((128, T), NEG, np.float32)
    id16 = np.eye(128, dtype=np.float32)
    return {
        "keys": k,
        "weff": weff.astype(ml_dtypes.bfloat16),
        "cbias": cbv,
        "bd2": bd2.astype(ml_dtypes.bfloat16),
        "bd3e": bd3e.astype(ml_dtypes.bfloat16),
        "bd3o": bd3o.astype(ml_dtypes.bfloat16),
        "b2rep": b2r,
        "masks": mperm,
        "negt": negt,
        "id16": id16.astype(ml_dtypes.bfloat16),
        "idf": id16,
    }


def kernel(queries, keys, key_masks, W1, b1, W2, b2, W3, b3):
    nc = _get_nc(NTT)
    in_maps = [make_core_inputs(queries, keys, key_masks, W1, b1, W2, b2,
                                W3, b3, core) for core in range(NCORES)]
    res = run_bass_kernel_spmd(nc, in_maps, list(range(NCORES)))
    outs = [res.results[c]["out"] for c in range(NCORES)]
    return np.concatenate(outs, axis=0).reshape(B, 1, D).astype(np.float32)
